# revision 16
# baseline (speedup 1.0000x reference)
"""LightGCN (3-layer) on 8 Trainium2 NeuronCores via Bass/Tile.

Strategy:
  - Reformulate: with w = edge_attr*mask, deg = segsum(w, col),
    hhat_0 = deg^-1/2 * x, hhat_{l+1}[c] = (1/deg[c]) * sum_{e: col=c} w_e * hhat_l[row_e]
    out[c] = 0.25*(x[c] + sum_l sqrt(deg[c])*hhat_l[c])
    This removes every per-edge dinv[row] gather; only hhat rows are gathered.
  - Destination-shard: core k owns node cols [k*12544, (k+1)*12544) = 98 blocks of 128.
  - Per (col-block g, row-quarter q) bucket, edges are packed into 128-slot
    sub-blocks (padded with w=0).  Sub-block counts are maxed across cores so
    all 8 cores run one SPMD program; per-core data differs only in inputs.
  - Gather: gpsimd.dma_gather (int16 idx, 256B rows) from the AllGathered
    hhat table, one gather per (chunk, row-quarter).
  - Segment-sum: one-hot lhsT built by one DVE tensor_scalar
    ((iota == col_local) * w), matmul-accumulated into PSUM [128 segs, 64].
  - 3 AllGathers of the 12544x64 local slices re-replicate hhat between layers.
  - All per-core constant tables (iota, col_local, w, x, gather indices) are
    packed into ONE input tensor -> one DMA -> one semaphore, because trn2
    instructions only support a small number of sync-wait commands.
"""

import numpy as np

N_NODES = 100000
D = 64
N_CORES = 8
BPC = 98                      # 128-col blocks per core
NLOC = BPC * 128              # 12544 local nodes per core
NPAD = N_CORES * NLOC         # 100352 padded node count
NQ = 4
QSIZE = NPAD // NQ            # 25088 rows per gather quarter (< 32768, int16-safe)
CG = 8                        # col-blocks per chunk (gather batching granularity)
ALPHA = 0.25
N_LAYERS = 3


def _preprocess(x, edge_attr, edge_index, edge_mask):
    """Bucket masked-in edges by (core, col-block, row-quarter); build per-core
    packed constant tables.  Returns per-core input dicts + static structure."""
    keep = np.asarray(edge_mask).astype(bool)
    row = np.asarray(edge_index[0])[keep].astype(np.int64)
    col = np.asarray(edge_index[1])[keep].astype(np.int64)
    w = np.asarray(edge_attr)[keep].astype(np.float32)

    core = col // NLOC
    g = (col % NLOC) >> 7
    q = row // QSIZE
    key = ((core * BPC + g) * NQ + q).astype(np.int64)
    order = np.argsort(key, kind="stable")
    row, col, w, key = row[order], col[order], w[order], key[order]
    counts = np.bincount(key, minlength=N_CORES * BPC * NQ).reshape(N_CORES, BPC, NQ)

    # Common sub-block structure: S[g, q] = max over cores of ceil(count/128)
    S = (-(-counts // 128)).max(axis=0)  # [BPC, NQ]

    # Global sub-block ordering: for chunk c, for q, for g in chunk, for s.
    chunks = [range(c0, min(c0 + CG, BPC)) for c0 in range(0, BPC, CG)]
    sb0 = np.zeros((BPC, NQ), np.int64)     # first global sub-block id of (g,q)
    gather_meta = []                        # per (chunk, q): (sb_base, n_subblocks)
    nxt = 0
    for blocks in chunks:
        per_q = []
        for qq in range(NQ):
            base = nxt
            for gg in blocks:
                sb0[gg, qq] = nxt
                nxt += S[gg, qq]
            per_q.append((base, nxt - base))
        gather_meta.append(per_q)
    SB_TOT = nxt

    bucket_starts = np.concatenate([[0], np.cumsum(counts.reshape(-1))])
    ins = []
    xf = np.asarray(x, np.float32)
    for k in range(N_CORES):
        nslot = SB_TOT * 128
        colloc = np.zeros(nslot, np.float32)
        wv = np.zeros(nslot, np.float32)
        idxv = np.zeros(nslot, np.int16)
        for gg in range(BPC):
            for qq in range(NQ):
                b = (k * BPC + gg) * NQ + qq
                lo, hi = bucket_starts[b], bucket_starts[b + 1]
                n = hi - lo
                if n == 0:
                    continue
                base = sb0[gg, qq] * 128
                slot = base + np.arange(n)
                colloc[slot] = (col[lo:hi] & 127).astype(np.float32)
                wv[slot] = w[lo:hi]
                idxv[slot] = (row[lo:hi] - qq * QSIZE).astype(np.int16)
        coltab = colloc.reshape(SB_TOT, 128).T.copy()
        wtab = wv.reshape(SB_TOT, 128).T.copy()
        # idx wrapped layout per (chunk, q) region: position i -> [i%16, off + i//16]
        idxtab = np.zeros((128, 8 * SB_TOT), np.int16)
        for per_q in gather_meta:
            for (base, ns) in per_q:
                if ns == 0:
                    continue
                seg = idxv[base * 128:(base + ns) * 128]
                wrapped = seg.reshape(ns * 8, 16).T  # [16, 8*ns]
                idxtab[:, 8 * base: 8 * (base + ns)] = np.tile(wrapped, (8, 1))
        xloc = np.zeros((NLOC, D), np.float32)
        lo = k * NLOC
        hi = min((k + 1) * NLOC, N_NODES)
        xloc[: hi - lo] = xf[lo:hi]
        xp = xloc.reshape(BPC, 128, D).transpose(1, 0, 2).reshape(128, BPC * D)
        iota = np.tile(np.arange(128, dtype=np.float32)[None, :], (128, 1))
        ctab = np.concatenate(
            [iota, coltab, wtab, xp, idxtab.view(np.float32)], axis=1)
        ins.append({"ctab": np.ascontiguousarray(ctab)})
    return ins, S, sb0, gather_meta, chunks, SB_TOT


def _build(S, sb0, gather_meta, chunks, SB_TOT, use_cc=True, core_id=0, phases=3, linearize=False):
    import concourse.bacc as bacc
    import concourse.mybir as mybir
    import concourse.tile as tile

    f32 = mybir.dt.float32
    i16 = mybir.dt.int16
    Alu = mybir.AluOpType
    Act = mybir.ActivationFunctionType

    # packed const layout (f32 words per partition)
    O_IOTA = 0
    O_COL = 128
    O_W = O_COL + SB_TOT
    O_X = O_W + SB_TOT
    O_IDX = O_X + BPC * D
    TOTW = O_IDX + 4 * SB_TOT

    nc = bacc.Bacc("TRN2", target_bir_lowering=False, debug=False,
                   num_devices=N_CORES)

    ct_in = nc.dram_tensor("ctab", [128, TOTW], f32, kind="ExternalInput")
    out_ext = nc.dram_tensor("outloc", [NLOC, D], f32, kind="ExternalOutput")

    hloc = [nc.dram_tensor(f"hloc{l}", [NLOC, D], f32) for l in range(N_LAYERS)]
    hag = [nc.dram_tensor(f"hag{l}", [NPAD, D], f32, addr_space="Shared")
           for l in range(N_LAYERS)]
    rg = [list(range(N_CORES))]

    nidx_regs = {}

    def nidx_reg(v):
        if v not in nidx_regs:
            nidx_regs[v] = nc.gpsimd.to_reg(v)
        return nidx_regs[v]

    # per-block sub-block lists: (q, s) pairs in chunk iteration order
    def block_subblocks(gg):
        out = []
        for qq in range(NQ):
            for s in range(S[gg, qq]):
                out.append((qq, s))
        return out

    with tile.TileContext(nc, linearize=linearize) as tc:
        with (
            tc.tile_pool(name="const", bufs=1) as constp,
            tc.tile_pool(name="big", bufs=1) as bigp,
            tc.tile_pool(name="dst", bufs=3) as dstp,
            tc.tile_pool(name="oh", bufs=6) as ohp,
            tc.tile_pool(name="psd", bufs=2, space="PSUM") as psdp,
            tc.tile_pool(name="ps", bufs=6, space="PSUM") as psp,
            tc.tile_pool(name="ev", bufs=3) as evp,
        ):
            ct = constp.tile([128, TOTW], f32)
            nc.sync.dma_start(ct[:], ct_in.ap())
            iota = ct[:, O_IOTA:O_IOTA + 128]
            coltab = ct[:, O_COL:O_COL + SB_TOT]
            wtab = ct[:, O_W:O_W + SB_TOT]
            xall = ct[:, O_X:O_X + BPC * D]
            idxtab = ct[:, O_IDX:O_IDX + 4 * SB_TOT].bitcast(i16)

            outacc = bigp.tile([128, BPC, D], f32)

            # ---- deg phase ----
            degt = bigp.tile([128, BPC], f32)
            for gg in range(BPC):
                sbs = block_subblocks(gg)
                if not sbs:
                    nc.vector.memset(degt[:, gg:gg + 1], 0.0)
                    continue
                psd = psdp.tile([128, 1], f32, tag="psd")
                for i, (qq, s) in enumerate(sbs):
                    sb = sb0[gg, qq] + s
                    oh = ohp.tile([128, 128], f32, tag="oh01")
                    nc.vector.tensor_scalar(
                        oh[:], iota, coltab[:, sb:sb + 1], None, op0=Alu.is_equal)
                    nc.tensor.matmul(psd[:], oh[:], wtab[:, sb:sb + 1],
                                     start=(i == 0), stop=(i == len(sbs) - 1))
                nc.vector.tensor_copy(degt[:, gg:gg + 1], psd[:])

            # ---- dinv / invdeg ----
            mt = bigp.tile([128, BPC], f32)
            nc.vector.tensor_scalar(mt[:], degt[:], 0.0, None, op0=Alu.is_gt)
            st = bigp.tile([128, BPC], f32)
            nc.vector.tensor_scalar(st[:], degt[:], 1e-30, None, op0=Alu.max)
            invraw = bigp.tile([128, BPC], f32)
            nc.vector.reciprocal(invraw[:], st[:])
            invdegt = bigp.tile([128, BPC], f32)
            nc.vector.tensor_tensor(invdegt[:], invraw[:], mt[:], op=Alu.mult)
            rs = bigp.tile([128, BPC], f32)
            nc.scalar.activation(rs[:], invraw[:], Act.Sqrt)
            dinvt = bigp.tile([128, BPC], f32)
            nc.vector.tensor_tensor(dinvt[:], rs[:], mt[:], op=Alu.mult)
            adinvt = bigp.tile([128, BPC], f32)
            nc.vector.tensor_scalar(adinvt[:], dinvt[:], ALPHA, None, op0=Alu.mult)

            if phases == 0:
                nc.vector.memset(outacc[:].rearrange("p g d -> p (g d)"), 0.0)
                nc.vector.tensor_copy(outacc[:, :, 0], degt[:])
                out_r0 = out_ext.ap().rearrange("(g p) d -> p g d", p=128)
                nc.sync.dma_start(out_r0, outacc[:])
                nc.compile()
                return nc

            # ---- out init + hhat0 ----
            nc.vector.tensor_scalar(
                outacc[:].rearrange("p g d -> p (g d)"), xall, ALPHA, None,
                op0=Alu.mult)
            h0all = bigp.tile([128, BPC, D], f32)
            for gg in range(BPC):
                nc.vector.tensor_scalar(
                    h0all[:, gg, :], xall[:, gg * D:(gg + 1) * D],
                    dinvt[:, gg:gg + 1], None, op0=Alu.mult)
            h0_r = hloc[0].ap().rearrange("(g p) d -> p g d", p=128)
            nc.sync.dma_start(h0_r, h0all[:])
            if use_cc:
                nc.gpsimd.collective_compute(
                    "AllGather", Alu.bypass, replica_groups=rg,
                    ins=[hloc[0].ap().opt()], outs=[hag[0].ap().opt()])
            else:
                nc.sync.dma_start(
                    hag[0].ap()[core_id * NLOC:(core_id + 1) * NLOC, :],
                    hloc[0].ap())

            # ---- layers ----
            for l in range(N_LAYERS):
                if l < N_LAYERS - 1:
                    hnall = bigp.tile([128, BPC, D], f32, tag=f"hnall{l}")
                else:
                    hnall = None
                for ci, blocks in enumerate(chunks):
                    dsts = {}
                    for qq in range(NQ):
                        base, ns = gather_meta[ci][qq]
                        if ns == 0:
                            continue
                        dstt = dstp.tile([128, ns, D], f32, tag=f"dst{qq}")
                        src = hag[l].ap()[qq * QSIZE:(qq + 1) * QSIZE, :]
                        # >512-idx dma_gather kills the device (NRT 101);
                        # split into <=4-sub-block pieces
                        for p0 in range(0, ns, 4):
                            pe_ = min(p0 + 4, ns)
                            npc = (pe_ - p0) * 128
                            idxs = idxtab[:, 8 * (base + p0): 8 * (base + pe_)]
                            nc.gpsimd.dma_gather(
                                dstt[:, p0:pe_, :], src, idxs, npc,
                                nidx_reg(npc), D, queue_num=0)
                        dsts[qq] = (dstt, base)
                    for gg in blocks:
                        sbs = block_subblocks(gg)
                        if not sbs:
                            if hnall is not None:
                                nc.vector.memset(hnall[:, gg, :], 0.0)
                            continue
                        ps = psp.tile([128, D], f32, tag="ps")
                        for i, (qq, s) in enumerate(sbs):
                            sb = sb0[gg, qq] + s
                            dstt, base = dsts[qq]
                            pos = sb - base
                            oh = ohp.tile([128, 128], f32, tag="ohw")
                            nc.vector.tensor_scalar(
                                oh[:], iota, coltab[:, sb:sb + 1],
                                wtab[:, sb:sb + 1], op0=Alu.is_equal, op1=Alu.mult)
                            nc.tensor.matmul(ps[:], oh[:], dstt[:, pos, :],
                                             start=(i == 0),
                                             stop=(i == len(sbs) - 1))
                        if hnall is not None:
                            nc.vector.tensor_scalar(
                                hnall[:, gg, :], ps[:],
                                invdegt[:, gg:gg + 1], None, op0=Alu.mult)
                        tmp = evp.tile([128, D], f32, tag="tmp")
                        nc.vector.tensor_scalar(tmp[:], ps[:],
                                                adinvt[:, gg:gg + 1], None,
                                                op0=Alu.mult)
                        nc.vector.tensor_tensor(
                            outacc[:, gg, :],
                            outacc[:, gg, :], tmp[:], op=Alu.add)
                if hnall is not None:
                    hn_r = hloc[l + 1].ap().rearrange("(g p) d -> p g d", p=128)
                    nc.sync.dma_start(hn_r, hnall[:])
                    if use_cc:
                        nc.gpsimd.collective_compute(
                            "AllGather", Alu.bypass, replica_groups=rg,
                            ins=[hloc[l + 1].ap().opt()],
                            outs=[hag[l + 1].ap().opt()])
                    else:
                        nc.sync.dma_start(
                            hag[l + 1].ap()[core_id * NLOC:(core_id + 1) * NLOC, :],
                            hloc[l + 1].ap())

            out_r = out_ext.ap().rearrange("(g p) d -> p g d", p=128)
            nc.sync.dma_start(out_r, outacc[:])
    nc.compile()
    return nc


_CACHE = {}


def kernel(x, edge_attr, edge_index, edge_mask, _want_profile=False):
    from concourse import bass_utils

    ins, S, sb0, gather_meta, chunks, SB_TOT = _preprocess(
        x, edge_attr, edge_index, edge_mask)
    ck = ("k", SB_TOT, S.tobytes())
    if ck not in _CACHE:
        _CACHE[ck] = _build(S, sb0, gather_meta, chunks, SB_TOT, linearize=False)
    nc = _CACHE[ck]
    res = bass_utils.run_bass_kernel_spmd(
        nc, ins, core_ids=list(range(N_CORES)), trace=_want_profile)
    out = np.concatenate([res.results[k]["outloc"] for k in range(N_CORES)],
                         axis=0)[:N_NODES]
    if _want_profile:
        return out.astype(np.float32), res
    return out.astype(np.float32)
